# revision 6
# baseline (speedup 1.0000x reference)
"""GCN layer (nn_GCNLayer) on 8 TRN2 NeuronCores via Bass/Tile.

Reference math (f32):
    A_hat  = A + I
    D      = A_hat.sum(axis=1)                  # = rowsum(A) + 1
    d      = 1/sqrt(D + 1e-10)
    out    = relu((d[:,None] * A_hat * d[None,:]) @ (X @ W))

Rewritten to avoid materializing A_norm:
    Ys     = d[:,None] * (X @ W)                                 # [N, C]
    out[r] = relu(d[r] * (A[r,:] @ Ys + Ys[r]))                  # +Ys[r] is the +I diag

Sharding (8 cores): rows of A ([N/8, N]) and X ([N/8, F]); W replicated.
Per core:
  phase 0: XW_loc = X_shard @ W (bf16 matmul, f32 accum)
  phase 1: stream A_shard f32 from HBM once; per [128, CH] chunk:
           - ScalarE cast f32->bf16 with free-dim accumulation => rowsums
           - PE transpose 128x128 bf16 tiles -> PSUM; drain to SBUF-resident A^T
  boundary: d_loc = 1/sqrt(rowsum+1+1e-10); Ys_loc = d_loc*XW_loc;
           AllGather(Ys_loc bf16) -> Ys [N, C]; DMA back to SBUF
  phase 2: per 128-row stripe: accumulate 64 matmuls lhsT=A^T tile, rhs=Ys tile
           into PSUM; += Ys_loc[stripe]; relu(d*psum) -> out
"""

import os
import sys

import numpy as np

sys.path.insert(0, "/opt/trn_rl_repo")

from contextlib import ExitStack

from concourse import bacc, bass, mybir, tile
from concourse.bass_utils import run_bass_kernel_spmd
from concourse.masks import make_identity

F32 = mybir.dt.float32
BF16 = mybir.dt.bfloat16
AF = mybir.ActivationFunctionType


def _ensure_axon_ntff_hook():
    """run_bass_kernel_spmd(trace=True) under axon imports
    antenv.axon_hooks, which the container's antenv stub lacks. Provide it
    via sys.modules, driving NTFF capture through libaxon_pjrt.so ctypes."""
    try:
        import antenv.axon_hooks  # noqa: F401

        return
    except ImportError:
        pass
    import contextlib
    import ctypes
    import types

    mod = types.ModuleType("antenv.axon_hooks")
    state = {"hook": None}

    def _build(so_path):
        if not os.path.exists(so_path):
            return None
        lib = ctypes.CDLL(so_path)
        if not hasattr(lib, "axon_start_nrt_profile"):
            return None
        lib.axon_start_nrt_profile.argtypes = [
            ctypes.POINTER(ctypes.c_int64),
            ctypes.c_size_t,
        ]
        lib.axon_start_nrt_profile.restype = ctypes.c_int64
        lib.axon_stop_nrt_profile.argtypes = [ctypes.c_char_p]
        lib.axon_stop_nrt_profile.restype = ctypes.c_int64

        @contextlib.contextmanager
        def _hook(output_dir, device_ids):
            import jax

            jax.devices()
            if device_ids:
                ids = (ctypes.c_int64 * len(device_ids))(*device_ids)
                rc = lib.axon_start_nrt_profile(ids, len(device_ids))
            else:
                rc = lib.axon_start_nrt_profile(None, 0)
            if rc != 0:
                raise RuntimeError(f"axon_start_nrt_profile rc={rc}")
            try:
                yield
            finally:
                n = lib.axon_stop_nrt_profile(str(output_dir).encode())
                if n < 0:
                    raise RuntimeError(f"axon_stop_nrt_profile rc={n}")

        return _hook

    def set_axon_ntff_profile_hook(hook):
        state["hook"] = hook

    def get_axon_ntff_profile_hook():
        if state["hook"] is None:
            state["hook"] = _build(
                os.environ.get("AXON_PJRT_SO", "/opt/axon/libaxon_pjrt.so")
            )
        return state["hook"]

    mod.set_axon_ntff_profile_hook = set_axon_ntff_profile_hook
    mod.get_axon_ntff_profile_hook = get_axon_ntff_profile_hook
    sys.modules["antenv.axon_hooks"] = mod
    try:
        import antenv

        antenv.axon_hooks = mod
    except ImportError:
        pass

N, FDIM, CDIM = 8192, 512, 256
NCORES = 8


def build(n=N, fdim=FDIM, cdim=CDIM, ncores=NCORES, ch=1024):
    """Build the SPMD Bass program (identical on every core)."""
    R = n // ncores      # rows per core
    S = R // 128         # 128-row stripes per core
    KT = n // 128        # contraction tiles
    NCH = n // ch        # chunks per stripe
    FT = fdim // 128
    KPC = KT // ncores   # k-tiles owned per core (= S)
    assert KPC == S

    nc = bacc.Bacc(
        "TRN2", target_bir_lowering=False, debug=False, num_devices=ncores
    )
    A_d = nc.dram_tensor("A", [R, n], F32, kind="ExternalInput").ap()
    X_d = nc.dram_tensor("X", [R, fdim], F32, kind="ExternalInput").ap()
    W_d = nc.dram_tensor("W", [fdim, cdim], F32, kind="ExternalInput").ap()
    out_d = nc.dram_tensor("out", [R, cdim], F32, kind="ExternalOutput").ap()
    ys_in_d = nc.dram_tensor("ys_in", [R, cdim], BF16).ap()
    ys_out_d = nc.dram_tensor("ys_out", [n, cdim], BF16, addr_space="Shared").ap()

    with tile.TileContext(nc) as tc, ExitStack() as ctx:
        const_pool = ctx.enter_context(tc.tile_pool(name="const", bufs=1))
        ident = const_pool.tile([128, 128], BF16)
        make_identity(nc, ident[:])

        # Persistent big tensors.
        at_pool = ctx.enter_context(tc.tile_pool(name="atp", bufs=1))
        # A^T bf16, stripe-major: slice (s, kt) at free offset (s*KT + kt)*128
        AT = at_pool.tile([128, S * KT * 128], BF16)
        ys_pool = ctx.enter_context(tc.tile_pool(name="ysp", bufs=1))
        ys_sb = ys_pool.tile([128, KT * cdim], BF16)   # Ys, kt-major

        small_pool = ctx.enter_context(tc.tile_pool(name="small", bufs=1))
        xw_f32 = small_pool.tile([128, S * cdim], F32)   # XW_loc then Ys_loc (in place)
        ysloc_bf = small_pool.tile([128, S * cdim], BF16)
        Dacc = small_pool.tile([128, S * NCH], F32)
        Dsum = small_pool.tile([128, S], F32)
        d_loc = small_pool.tile([128, S], F32)

        # ---- Phase 0: XW_loc = X_shard @ W (bf16) ----
        with tc.tile_pool(name="ph0", bufs=2) as ph0, \
             tc.tile_pool(name="ph0c", bufs=1) as ph0c, \
             tc.tile_pool(name="ph0ps", bufs=2, space="PSUM") as ph0ps:
            w_f32 = ph0c.tile([128, FT * cdim], F32)
            w_bf = ph0c.tile([128, FT * cdim], BF16)
            for f in range(FT):
                nc.sync.dma_start(
                    w_f32[:, f * cdim:(f + 1) * cdim],
                    W_d[f * 128:(f + 1) * 128, :],
                )
            nc.vector.tensor_copy(w_bf[:], w_f32[:])

            xT = ph0c.tile([128, S * FT * 128], BF16)  # X^T tiles, (s, f)
            for s in range(S):
                x_f32 = ph0.tile([128, fdim], F32)
                nc.sync.dma_start(x_f32[:], X_d[s * 128:(s + 1) * 128, :])
                x_bf = ph0.tile([128, fdim], BF16)
                nc.vector.tensor_copy(x_bf[:], x_f32[:])
                pxt = ph0ps.tile([128, fdim], BF16)
                for f in range(FT):
                    nc.tensor.transpose(
                        pxt[:, f * 128:(f + 1) * 128],
                        x_bf[:, f * 128:(f + 1) * 128],
                        ident[:],
                    )
                nc.scalar.copy(
                    xT[:, (s * FT) * 128:(s * FT + FT) * 128], pxt[:]
                )
            for s in range(S):
                pxw = ph0ps.tile([128, cdim], F32)
                for f in range(FT):
                    nc.tensor.matmul(
                        pxw[:],
                        lhsT=xT[:, (s * FT + f) * 128:(s * FT + f + 1) * 128],
                        rhs=w_bf[:, f * cdim:(f + 1) * cdim],
                        start=(f == 0),
                        stop=(f == FT - 1),
                    )
                nc.vector.tensor_copy(xw_f32[:, s * cdim:(s + 1) * cdim], pxw[:])

        # ---- Phase 1: stream A, cast+rowsum, transpose into resident A^T ----
        TPC = ch // 128          # transposes per chunk
        GRP = 4                  # transposes per PSUM bank / drain
        with tc.tile_pool(name="ast", bufs=3) as ast, \
             tc.tile_pool(name="abf", bufs=3) as abf, \
             tc.tile_pool(name="tps", bufs=3, space="PSUM") as tps:
            for s in range(S):
                for c in range(NCH):
                    a_ch = ast.tile([128, ch], F32)
                    nc.sync.dma_start(
                        a_ch[:], A_d[s * 128:(s + 1) * 128, c * ch:(c + 1) * ch]
                    )
                    a_bf = abf.tile([128, ch], BF16)
                    i = s * NCH + c
                    nc.scalar.activation(
                        a_bf[:], a_ch[:], AF.Copy, accum_out=Dacc[:, i:i + 1]
                    )
                    for g in range(TPC // GRP):
                        pt = tps.tile([128, GRP * 128], BF16)
                        for t in range(GRP):
                            nc.tensor.transpose(
                                pt[:, t * 128:(t + 1) * 128],
                                a_bf[:, (g * GRP + t) * 128:(g * GRP + t + 1) * 128],
                                ident[:],
                            )
                        kt0 = c * TPC + g * GRP
                        dst = AT[:, (s * KT + kt0) * 128:(s * KT + kt0 + GRP) * 128]
                        if g % 2 == 0:
                            nc.vector.tensor_copy(dst, pt[:])
                        else:
                            nc.scalar.copy(dst, pt[:])

        # ---- Boundary: d, Ys_loc, AllGather ----
        nc.vector.tensor_reduce(
            Dsum[:],
            Dacc[:].rearrange("p (s c) -> p s c", s=S),
            axis=mybir.AxisListType.X,
            op=mybir.AluOpType.add,
        )
        # Dsq = sqrt(D + 1 + 1e-10); d = 1/Dsq
        Dsq = small_pool.tile([128, S], F32)
        bias1 = small_pool.tile([128, 1], F32)
        nc.gpsimd.memset(bias1[:], 1.0 + 1e-10)
        nc.scalar.activation(Dsq[:], Dsum[:], AF.Sqrt, bias=bias1[:])
        nc.vector.reciprocal(d_loc[:], Dsq[:])
        for s in range(S):
            nc.vector.tensor_scalar_mul(
                xw_f32[:, s * cdim:(s + 1) * cdim],
                xw_f32[:, s * cdim:(s + 1) * cdim],
                d_loc[:, s:s + 1],
            )
        nc.vector.tensor_copy(ysloc_bf[:], xw_f32[:])
        nc.sync.dma_start(
            ys_in_d.rearrange("(s p) c -> p s c", p=128),
            ysloc_bf[:].rearrange("p (s c) -> p s c", s=S),
        )
        nc.gpsimd.collective_compute(
            "AllGather",
            mybir.AluOpType.bypass,
            replica_groups=[list(range(ncores))],
            ins=[ys_in_d],
            outs=[ys_out_d],
        )
        # DMA Ys back: 8 chunks of KT/8 k-tiles each
        KCH = KT // 8
        for b in range(8):
            nc.sync.dma_start(
                ys_sb[:, b * KCH * cdim:(b + 1) * KCH * cdim].rearrange(
                    "p (k c) -> p k c", k=KCH
                ),
                ys_out_d[b * KCH * 128:(b + 1) * KCH * 128, :].rearrange(
                    "(k p) c -> p k c", p=128
                ),
            )

        # ---- Phase 2: out[s] = relu(d * (A_shard @ Ys + Ys_loc[s])) ----
        with tc.tile_pool(name="ops", bufs=2, space="PSUM") as ops, \
             tc.tile_pool(name="outp", bufs=3) as outp:
            for s in range(S):
                po = ops.tile([128, cdim], F32)
                for kt in range(KT):
                    nc.tensor.matmul(
                        po[:],
                        lhsT=AT[:, (s * KT + kt) * 128:(s * KT + kt + 1) * 128],
                        rhs=ys_sb[:, kt * cdim:(kt + 1) * cdim],
                        start=(kt == 0),
                        stop=(kt == KT - 1),
                    )
                nc.vector.tensor_add(
                    po[:], po[:], xw_f32[:, s * cdim:(s + 1) * cdim]
                )
                ot = outp.tile([128, cdim], F32)
                nc.scalar.activation(ot[:], po[:], AF.Relu, scale=d_loc[:, s:s + 1])
                nc.sync.dma_start(out_d[s * 128:(s + 1) * 128, :], ot[:])

    nc.compile()
    return nc


_NC_CACHE = {}


def _get_nc(key=(N, FDIM, CDIM, NCORES)):
    if key not in _NC_CACHE:
        _NC_CACHE[key] = build(*key)
    return _NC_CACHE[key]


def kernel(X, A, W, trace=False, **kw):
    X = np.ascontiguousarray(np.asarray(X, dtype=np.float32))
    A = np.ascontiguousarray(np.asarray(A, dtype=np.float32))
    W = np.ascontiguousarray(np.asarray(W, dtype=np.float32))
    n = A.shape[0]
    ncores = NCORES
    R = n // ncores
    if trace:
        _ensure_axon_ntff_hook()
    nc = _get_nc((n, X.shape[1], W.shape[1], ncores))
    in_maps = [
        {
            "A": A[i * R:(i + 1) * R],
            "X": X[i * R:(i + 1) * R],
            "W": W,
        }
        for i in range(ncores)
    ]
    res = run_bass_kernel_spmd(nc, in_maps, list(range(ncores)), trace=trace, **kw)
    out = np.concatenate([res.results[i]["out"] for i in range(ncores)], axis=0)
    if trace:
        return out, res
    return out
